# revision 19
# baseline (speedup 1.0000x reference)
"""Trainium2 Bass kernel for nn_MultiHeadAttention_63986422775834.

Computation (see harness reference):
    q = x @ Wq + bq; k = x @ Wk + bk; v = x @ Wv + bv          # [N, D]
    group rows by 8: scores[b,h,g] = q[8b+h] . k[8b+g] / sqrt(D)
    w = softmax(scores, axis=-1);  out[8b+h] = sum_g w[b,h,g] * v[8b+g]

Algebraic restructuring: scores[i,j] = x_i (Wq Wk^T) x_j^T + x_i.(Wq bk)
+ x_j.(Wk bq) + bq.bk.  The x_i.(Wq bk) and bq.bk terms are constant
along the softmax axis j and drop out; so with A = Wq Wk^T and
beta = Wk bq:
    scores[i,j] = u_i . x_j + b_j,   u = x @ A,   b = x @ beta
This replaces the two q/k projection GEMMs (2 x 2048^3 MACs/core) with
ONE GEMM u = x @ A, plus a SHARDED computation of A: each of the 8
cores computes a 256-row slice of A (Wq-slice^T x Wk^T) and the slices
are AllGather'd (4 column-group collectives, ~7us each, fully hidden
under compute).  bv passes through softmax (rows sum to 1), so it is
folded into the V tiles and O results DMA straight from PSUM.

Sharding: data-parallel over rows across 8 NeuronCores (2048 rows each;
row groups of 8 never cross a shard boundary).  Wk/Wv replicated;
Wq distributed as per-core 256-row slices.

Per-core schedule (bf16 matmuls, fp32 accumulate):
  startup: per quarter g: Wk slabs -> WkT (PE transpose); A-slice chunk
           GEMM -> bounce -> AllGather_g; x block g -> xT; V c0 block
           (Wv cols 0:256 + beta col -> per-row b).
  pass u:  stream gathered-A chunks; uT = A^T-oriented GEMM (m on
           partitions); S partials accumulated in SBUF; the last chunk's
           PSUM chain appends the rank-1 ones (x) b^T bias term.
  mid:     masked softmax over 8x8 diagonal blocks, PE-transpose of the
           weights, O c0 from the resident V c0 tiles.
  pass 2:  stream Wv c1..c7; V = x @ Wv + bv; O = w @ V; PSUM->DRAM out.
"""

import sys

sys.path.insert(0, "/opt/trn_rl_repo")

import numpy as np
import ml_dtypes

import concourse.mybir as mybir
import concourse.tile as tile
from concourse import bacc
from concourse.bass_utils import run_bass_kernel_spmd

# problem shape (hardcoded per contract)
N_FULL = 16384
D = 2048
H = 8
N_CORES = 8
R = N_FULL // N_CORES  # rows per core = 2048
P = 128
KO = D // P  # 16 k-subtiles along d_in
SL = D // N_CORES  # A rows computed per core = 256
SCALE = 1.0 / np.sqrt(np.float32(D))

BF16 = mybir.dt.bfloat16
F32 = mybir.dt.float32

BLOCKS = [(0, 512), (512, 512), (1024, 512), (1536, 512)]
N_SUB = R // P  # 16 row subtiles
MC = 4  # AllGather column groups of 512
ACH = 8  # a-chunk width 256 -> 8 chunks, 2 m-tiles each
V_CHUNKS = [(256 * c, 256) for c in range(1, 8)]  # c1..c7 (c0 special)


def build_program():
    nc = bacc.Bacc("TRN2", target_bir_lowering=False, debug=False, num_devices=N_CORES)

    xs = nc.dram_tensor("xs", [R, D], F32, kind="ExternalInput")
    Wk = nc.dram_tensor("Wk", [D, D], F32, kind="ExternalInput")
    Wv = nc.dram_tensor("Wv", [D, D], F32, kind="ExternalInput")
    wq_sl = nc.dram_tensor("wq_sl", [SL, D], F32, kind="ExternalInput")
    beta_col = nc.dram_tensor("beta_col", [P, KO], F32, kind="ExternalInput")
    bvb = nc.dram_tensor("bvb", [P, D], BF16, kind="ExternalInput")
    maskt = nc.dram_tensor("maskt", [P, P], F32, kind="ExternalInput")
    ident = nc.dram_tensor("ident", [P, P], BF16, kind="ExternalInput")
    ones1 = nc.dram_tensor("ones1", [1, P], BF16, kind="ExternalInput")
    out = nc.dram_tensor("out", [R, D], F32, kind="ExternalOutput")

    # d_in-major view of Wv: w[p, ko, n] = Wv[ko*128+p, n]
    wv_ap = Wv[:].rearrange("(ko p) n -> p ko n", p=P)

    from contextlib import ExitStack

    with tile.TileContext(nc) as tc:
        with ExitStack() as stack:
            pool = lambda *a, **kw: stack.enter_context(tc.tile_pool(*a, **kw))
            const = pool(name="const", bufs=1)
            xT_pool = pool(name="xT", bufs=1)
            big = pool(name="big", bufs=2)  # WkT halves -> uT chunks
            apool = pool(name="achunk", bufs=2)  # wqT_sl -> A chunks
            wvpool = pool(name="wv", bufs=2)
            v0pool = pool(name="v0", bufs=1)
            sacc = pool(name="sacc", bufs=1)
            wtmp = pool(name="wtmp", bufs=2)
            wkbp = pool(name="wkb", bufs=2)
            phA = pool(name="phA", bufs=2)
            aoutp = pool(name="aout", bufs=2)
            vpool = pool(name="vpool", bufs=2)
            opool = pool(name="opool", bufs=2)
            soft = pool(name="soft", bufs=2)
            dram = pool(name="dram", bufs=1, space="DRAM")
            ps_big = pool(name="ps_big", bufs=3, space="PSUM")
            ps_s = pool(name="ps_s", bufs=2, space="PSUM")
            ps_t = pool(name="ps_t", bufs=2, space="PSUM")
            ps_warm = pool(name="ps_warm", bufs=1, space="PSUM")
            # --- constants ---
            mask_sb = const.tile([P, P], F32)
            nc.sync.dma_start(mask_sb, maskt[:])
            ident_sb = const.tile([P, P], BF16)
            nc.sync.dma_start(ident_sb, ident[:])
            ones_sb = const.tile([1, P], BF16)
            nc.sync.dma_start(ones_sb, ones1[:])
            bv_sb = const.tile([P, D], BF16)
            nc.sync.dma_start(bv_sb, bvb[:])
            beta_f = const.tile([P, KO], F32)
            nc.sync.dma_start(beta_f, beta_col[:])
            beta_bf = const.tile([P, KO], BF16)
            nc.vector.tensor_copy(beta_bf, beta_f)

            # HAM warm-up: dependency-free matmuls keep the PE clock gate
            # at full rate through the DMA-bound startup window.
            for _ in range(60):
                wps = ps_warm.tile([P, P], F32, tag="warm", name="wps")
                nc.tensor.matmul(wps, lhsT=ident_sb, rhs=ident_sb, start=True, stop=True)

            # --- persistent SBUF intermediates ---
            # xT[bi][p, ko, r] = x[row0 + r, ko*128 + p]  (bf16)
            xT = [
                xT_pool.tile([P, KO, nrows], BF16, name=f"xT{bi}")
                for bi, (_, nrows) in enumerate(BLOCKS)
            ]
            # WkT halves: wkT[h][p, otl, d2] = Wk[d2, (h*8+otl)*128 + p]
            wkT = [big.tile([P, 8, D], BF16, tag="big", name=f"wkT{h}") for h in range(2)]
            # wqT_sl[p, ot, j] = wq_sl[j, ot*128 + p]
            wqT_sl = apool.tile([P, KO, SL], BF16, tag="ach", name="wqT_sl")
            # S accumulator + softmax-weight transpose + b staging
            S_all = sacc.tile([P, N_SUB, P], F32, name="S_all")
            wT_all = sacc.tile([P, N_SUB, P], BF16, name="wT_all")
            b_all = sacc.tile([P, N_SUB], BF16, name="b_all")
            # single-partition row: matmul operands must start at partition 0
            bT_flat = sacc.tile([1, N_SUB * P], BF16, name="bT_flat")
            # V c0 resident tiles (cols 0:256 of V, bias folded in)
            v0_all = v0pool.tile([P, N_SUB, 256], BF16, name="v0_all")

            # DRAM bounce buffers for the A AllGathers
            ag_in = [
                dram.tile([SL, 512], BF16, name=f"ag_in{g}") for g in range(MC)
            ]
            ag_out = [
                dram.tile([D, 512], BF16, addr_space="Shared", name=f"ag_out{g}")
                for g in range(MC)
            ]
            ag_out_ap = [
                ag_out[g][:].rearrange("(kt p) m -> p kt m", p=P) for g in range(MC)
            ]

            # --- Wq slice load + transpose (2MB; DMA'd first) ---
            for j in range(2):
                for s in range(4):
                    xt = phA.tile([P, 512], F32, tag="xt", name="wq_f")
                    nc.sync.dma_start(
                        xt, wq_sl[j * P : (j + 1) * P, s * 512 : (s + 1) * 512]
                    )
                    xb = phA.tile([P, 512], BF16, tag="xb", name="wq_b")
                    nc.vector.tensor_copy(xb, xt)
                    for t in range(4):
                        ot = s * 4 + t
                        pst = ps_t.tile([P, P], BF16, tag="tr", name="pst")
                        nc.tensor.transpose(pst, xb[:, t * P : (t + 1) * P], ident_sb)
                        nc.vector.tensor_copy(
                            wqT_sl[:, ot, j * P : (j + 1) * P], pst
                        )

            # --- Wv c0 chunk (cols 0:256) + beta column -> width 257 ---
            wv0 = wvpool.tile([P, KO, 257], BF16, tag="wv", name="wv0")
            for k0 in range(0, KO, 2):
                tmp = wtmp.tile([P, 2, 256], F32, tag="wtmp", name="wv0_tmp")
                nc.sync.dma_start(tmp, wv_ap[:, k0 : k0 + 2, 0:256])
                nc.vector.tensor_copy(wv0[:, k0 : k0 + 2, 0:256], tmp)
            nc.vector.tensor_copy(wv0[:, :, 256], beta_bf)

            # --- startup quarters: WkT, A-slice chunk, AllGather, xT, V c0 ---
            def wk_quarter(g):
                # 4 slabs of 128 Wk rows -> transposes into wkT
                for sl in range(4):
                    d2_0 = 512 * g + 128 * sl
                    for qq in range(4):
                        wkf = wtmp.tile([P, 2, 256], F32, tag="wtmp", name="wkf")
                        wkf2 = wkf[:].rearrange("p a b -> p (a b)")
                        nc.sync.dma_start(
                            wkf2, Wk[d2_0 : d2_0 + P, qq * 512 : (qq + 1) * 512]
                        )
                        wkb = wkbp.tile([P, 512], BF16, tag="wkb", name="wkb")
                        nc.vector.tensor_copy(wkb, wkf2)
                        for t in range(4):
                            ot = qq * 4 + t
                            pst = ps_t.tile([P, P], BF16, tag="tr", name="pst")
                            nc.tensor.transpose(
                                pst, wkb[:, t * P : (t + 1) * P], ident_sb
                            )
                            nc.vector.tensor_copy(
                                wkT[ot // 8][:, ot % 8, d2_0 : d2_0 + P], pst
                            )

            def a_slice_chunk(g):
                # A[si, 512g:512g+512] = wqT_sl^T @ WkT chunk; bounce + gather
                for db in range(2):
                    psA = ps_big.tile([P, 512], F32, tag="ps_big", name="psA")
                    for ot in range(KO):
                        nc.tensor.matmul(
                            psA,
                            lhsT=wqT_sl[:, ot, db * P : (db + 1) * P],
                            rhs=wkT[ot // 8][:, ot % 8, g * 512 : (g + 1) * 512],
                            start=(ot == 0),
                            stop=(ot == KO - 1),
                        )
                    for hh in range(2):
                        aob = aoutp.tile([P, 256], BF16, tag="aout", name="aob")
                        nc.scalar.activation(
                            aob,
                            psA[:, hh * 256 : (hh + 1) * 256],
                            mybir.ActivationFunctionType.Identity,
                        )
                        nc.sync.dma_start(
                            ag_in[g][db * P : (db + 1) * P, hh * 256 : (hh + 1) * 256],
                            aob,
                        )
                nc.gpsimd.collective_compute(
                    "AllGather",
                    mybir.AluOpType.bypass,
                    replica_groups=[list(range(N_CORES))],
                    ins=[ag_in[g].opt()],
                    outs=[ag_out[g].opt()],
                )

            def phase_a_block(bi):
                row0, nrows = BLOCKS[bi]
                for s in range(4):  # 512-col strips of d_in
                    for rt in range(nrows // P):
                        r0 = row0 + rt * P
                        xt = phA.tile([P, 512], F32, tag="xt", name="xt")
                        nc.sync.dma_start(
                            xt, xs[r0 : r0 + P, s * 512 : (s + 1) * 512]
                        )
                        xb = phA.tile([P, 512], BF16, tag="xb", name="xb")
                        nc.vector.tensor_copy(xb, xt)
                        for t in range(4):
                            kt = s * 4 + t
                            pst = ps_t.tile([P, P], BF16, tag="tr", name="pst")
                            nc.tensor.transpose(
                                pst, xb[:, t * P : (t + 1) * P], ident_sb
                            )
                            nc.vector.tensor_copy(
                                xT[bi][:, kt, rt * P : (rt + 1) * P], pst
                            )

            def v0_block(bi):
                # V c0 (+bias) and the per-row b = x.beta (psum col 256)
                for rs in range(4):
                    i = bi * 4 + rs
                    psv = ps_big.tile([P, 512], F32, tag="ps_big", name="psv")[:, :257]
                    for kt in range(KO):
                        nc.tensor.matmul(
                            psv,
                            lhsT=xT[bi][:, kt, rs * P : (rs + 1) * P],
                            rhs=wv0[:, kt, :],
                            start=(kt == 0),
                            stop=(kt == KO - 1),
                        )
                    nc.vector.tensor_copy(b_all[:, i : i + 1], psv[:, 256:257])
                    nc.vector.tensor_add(
                        v0_all[:, i, :], psv[:, 0:256], bv_sb[:, 0:256]
                    )

            # ring-order note: phase_a_block(g) is emitted before
            # a_slice_chunk(g) so the A-bounce DMA (gated on the A-chunk
            # matmuls) sits behind the x loads in the sync DMA ring.
            for g in range(MC):
                wk_quarter(g)
                phase_a_block(g)
                a_slice_chunk(g)
                v0_block(g)

            # b columns -> partition-0 rows (rhs for the rank-1 bias matmul)
            for i in range(N_SUB):
                pst = ps_t.tile([P, P], BF16, tag="tr", name="pst_b")
                nc.tensor.transpose(
                    pst[:1, :], b_all[:, i : i + 1], ident_sb
                )
                nc.vector.tensor_copy(
                    bT_flat[:, i * P : (i + 1) * P], pst[:1, :]
                )

            # --- pass u: uT GEMM + S partial accumulation ---
            def load_a_chunk(ach):
                g, half = ach // 2, ach % 2
                dst = apool.tile([P, KO, 256], BF16, tag="ach", name="a_sb")
                nc.sync.dma_start(
                    dst, ag_out_ap[g][:, :, half * 256 : (half + 1) * 256]
                )
                return dst

            a_tiles = {0: load_a_chunk(0), 1: load_a_chunk(1)}
            uT = [None, None]  # big-pool slots, 8 m-tiles each

            for ach in range(ACH):
                if ach + 2 < ACH:
                    a_tiles[ach + 2] = load_a_chunk(ach + 2)
                a_sb = a_tiles.pop(ach)
                uh = ach // 4  # which uT half
                if ach % 4 == 0:
                    uT[uh] = big.tile([P, 8, R], BF16, tag="big", name=f"uT{uh}")
                for ml in range(2):
                    m_local = (ach % 4) * 2 + ml
                    for bi, (row0, nrows) in enumerate(BLOCKS):
                        psu = ps_big.tile([P, 512], F32, tag="ps_big", name="psu")
                        for kt in range(KO):
                            nc.tensor.matmul(
                                psu,
                                lhsT=a_sb[:, kt, ml * P : (ml + 1) * P],
                                rhs=xT[bi][:, kt, :],
                                start=(kt == 0),
                                stop=(kt == KO - 1),
                            )
                        nc.scalar.activation(
                            uT[uh][:, m_local, row0 : row0 + nrows],
                            psu,
                            mybir.ActivationFunctionType.Identity,
                        )
                # S partials for the 2 m-tiles of this chunk
                last = ach == ACH - 1
                for i in range(N_SUB):
                    bi, rs = i // 4, i % 4
                    pss = ps_s.tile([P, P], F32, tag="pss", name="pss")
                    for ml in range(2):
                        m_local = (ach % 4) * 2 + ml
                        mt = ach * 2 + ml
                        nc.tensor.matmul(
                            pss,
                            lhsT=uT[uh][:, m_local, i * P : (i + 1) * P],
                            rhs=xT[bi][:, mt, rs * P : (rs + 1) * P],
                            start=(ml == 0),
                            stop=(ml == 1 and not last),
                        )
                    if last:
                        # rank-1 bias: S[i,j] += 1_i * b_j
                        nc.tensor.matmul(
                            pss,
                            lhsT=ones_sb,
                            rhs=bT_flat[:, i * P : (i + 1) * P],
                            start=False,
                            stop=True,
                        )
                    if ach == 0:
                        nc.vector.tensor_copy(S_all[:, i, :], pss)
                    else:
                        nc.vector.tensor_add(S_all[:, i, :], S_all[:, i, :], pss)

            # --- softmax + O c0 ---
            def emit_softmax(i):
                tmask = soft.tile([P, P], F32, tag="tmask")
                nc.vector.tensor_add(tmask, S_all[:, i, :], mask_sb)
                e = soft.tile([P, P], F32, tag="e")
                ssum = soft.tile([P, 1], F32, tag="ssum")
                nc.scalar.activation(
                    e, tmask, mybir.ActivationFunctionType.Exp,
                    scale=float(SCALE), accum_out=ssum,
                )
                rcp = soft.tile([P, 1], F32, tag="rcp")
                nc.vector.reciprocal(rcp, ssum)
                wsb = soft.tile([P, P], BF16, tag="wsb")
                nc.vector.tensor_scalar_mul(wsb, e, rcp)
                pstw = ps_t.tile([P, P], BF16, tag="tr", name="pstw")
                nc.tensor.transpose(pstw, wsb, ident_sb)
                nc.vector.tensor_copy(wT_all[:, i, :], pstw)

            for i in range(N_SUB):
                emit_softmax(i)
                pso = ps_big.tile([P, 512], F32, tag="ps_big", name="pso0")[:, :256]
                nc.tensor.matmul(
                    pso, lhsT=wT_all[:, i, :], rhs=v0_all[:, i, :],
                    start=True, stop=True,
                )
                o_sb = opool.tile([P, 256], F32, tag="o", name="o_sb0")
                nc.vector.tensor_copy(o_sb, pso)
                nc.sync.dma_start(out[i * P : (i + 1) * P, 0:256], o_sb)

            # --- pass 2: V chunks c1..c7 + O ---
            def load_wv_chunk(c):
                col0, width = V_CHUNKS[c]
                dst = wvpool.tile([P, KO, 257], BF16, tag="wv", name="wv_sb")
                for k0 in range(0, KO, 2):
                    tmp = wtmp.tile([P, 2, 256], F32, tag="wtmp", name="wv_tmp")
                    nc.sync.dma_start(tmp, wv_ap[:, k0 : k0 + 2, col0 : col0 + width])
                    nc.vector.tensor_copy(dst[:, k0 : k0 + 2, 0:width], tmp)
                return dst

            wv_tiles = {0: load_wv_chunk(0)}
            pending_o = None  # (v_sb, i, col0, width)

            def emit_o(v_sb, i, col0, width):
                pso = ps_big.tile([P, 512], F32, tag="ps_big", name="pso")[:, :width]
                nc.tensor.matmul(
                    pso, lhsT=wT_all[:, i, :], rhs=v_sb, start=True, stop=True
                )
                o_sb = opool.tile([P, 256], F32, tag="o", name="o_sb")[:, :width]
                nc.vector.tensor_copy(o_sb, pso)
                nc.sync.dma_start(out[i * P : (i + 1) * P, col0 : col0 + width], o_sb)

            for c in range(len(V_CHUNKS)):
                col0, width = V_CHUNKS[c]
                if c + 1 < len(V_CHUNKS) and (c + 1) not in wv_tiles:
                    wv_tiles[c + 1] = load_wv_chunk(c + 1)
                wv_sb = wv_tiles.pop(c)
                for bi, (row0, nrows) in enumerate(BLOCKS):
                    for rs in range(4):
                        i = bi * 4 + rs
                        psv = ps_big.tile([P, 512], F32, tag="ps_big", name="psv2")[
                            :, :width
                        ]
                        for kt in range(KO):
                            nc.tensor.matmul(
                                psv,
                                lhsT=xT[bi][:, kt, rs * P : (rs + 1) * P],
                                rhs=wv_sb[:, kt, 0:width],
                                start=(kt == 0),
                                stop=(kt == KO - 1),
                            )
                        v_sb = vpool.tile([P, 256], BF16, tag="v", name="v_sb")[
                            :, :width
                        ]
                        nc.vector.tensor_add(
                            v_sb, psv, bv_sb[:, col0 : col0 + width]
                        )
                        if pending_o is not None:
                            emit_o(*pending_o)
                        pending_o = (v_sb, i, col0, width)
            if pending_o is not None:
                emit_o(*pending_o)
                pending_o = None

    nc.compile()
    return nc


_CACHED = {}


def host_constants():
    mask = np.full((P, P), -1e9, dtype=np.float32)
    for g in range(P // H):
        mask[g * H : (g + 1) * H, g * H : (g + 1) * H] = 0.0
    identity = np.eye(P, dtype=ml_dtypes.bfloat16)
    ones_row = np.ones((1, P), dtype=ml_dtypes.bfloat16)
    return mask, identity, ones_row


def make_in_maps(x, Wq, bq, Wk, bk, Wv, bv):
    x = np.ascontiguousarray(np.asarray(x, dtype=np.float32))
    Wq = np.ascontiguousarray(np.asarray(Wq, dtype=np.float32))
    Wk = np.ascontiguousarray(np.asarray(Wk, dtype=np.float32))
    Wv = np.ascontiguousarray(np.asarray(Wv, dtype=np.float32))
    bq = np.asarray(bq, dtype=np.float32)
    bv = np.asarray(bv, dtype=np.float32)

    mask, identity, ones_row = host_constants()
    beta = Wk @ bq  # [D]; the surviving score-bias term is b = x @ beta
    beta_col = np.ascontiguousarray(beta.reshape(KO, P).T)
    bvb = np.ascontiguousarray(
        np.broadcast_to(bv.astype(ml_dtypes.bfloat16), (P, D))
    )

    in_maps = []
    for i in range(N_CORES):
        in_maps.append(
            {
                "xs": x[i * R : (i + 1) * R],
                "Wk": Wk,
                "Wv": Wv,
                "wq_sl": np.ascontiguousarray(Wq[i * SL : (i + 1) * SL]),
                "beta_col": beta_col,
                "bvb": bvb,
                "maskt": mask,
                "ident": identity,
                "ones1": ones_row,
            }
        )
    return in_maps


def kernel(x, Wq, bq, Wk, bk, Wv, bv):
    if "nc" not in _CACHED:
        _CACHED["nc"] = build_program()
    nc = _CACHED["nc"]
    in_maps = make_in_maps(x, Wq, bq, Wk, bk, Wv, bv)
    res = run_bass_kernel_spmd(nc, in_maps, list(range(N_CORES)))
    return np.concatenate([res.results[i]["out"] for i in range(N_CORES)], axis=0)


# revision 30
# speedup vs baseline: 1.1759x; 1.1759x over previous
"""Trainium2 Bass kernel for nn_MultiHeadAttention_63986422775834.

Computation (see harness reference):
    q = x @ Wq + bq; k = x @ Wk + bk; v = x @ Wv + bv          # [N, D]
    group rows by 8: scores[b,h,g] = q[8b+h] . k[8b+g] / sqrt(D)
    w = softmax(scores, axis=-1);  out[8b+h] = sum_g w[b,h,g] * v[8b+g]

Algebraic restructuring: scores[i,j] = x_i (Wq Wk^T) x_j^T + x_i.(Wq bk)
+ x_j.(Wk bq) + bq.bk.  The x_i.(Wq bk) and bq.bk terms are constant
along the softmax axis j and drop out; so with A = Wq Wk^T and
beta = Wk bq:
    scores[i,j] = u_i . x_j + b_j,   u = x @ A,   b = x @ beta
This replaces the two q/k projection GEMMs (2 x 2048^3 MACs/core) with
ONE GEMM u = x @ A, plus a SHARDED computation of A: each of the 8
cores computes a 256-row slice of A (Wq-slice^T x Wk^T) and the slices
are AllGather'd (4 column-group collectives, ~7us each, fully hidden
under compute).  bv passes through softmax (rows sum to 1), so it is
folded into the V tiles and O results DMA straight from PSUM.

Sharding: data-parallel over rows across 8 NeuronCores (2048 rows each;
row groups of 8 never cross a shard boundary).  Wk/Wv replicated;
Wq distributed as per-core 256-row slices.

Per-core schedule (bf16 matmuls, fp32 accumulate):
  startup: per quarter g: Wk slabs -> WkT (PE transpose); A-slice chunk
           GEMM -> bounce -> AllGather_g; x block g -> xT; V c0 block
           (Wv cols 0:256 + beta col -> per-row b).
  pass u:  stream gathered-A chunks; uT = A^T-oriented GEMM (m on
           partitions); S partials accumulated in SBUF; the last chunk's
           PSUM chain appends the rank-1 ones (x) b^T bias term.
  mid:     masked softmax over 8x8 diagonal blocks, PE-transpose of the
           weights, O c0 from the resident V c0 tiles.
  pass 2:  stream Wv c1..c7; V = x @ Wv + bv; O = w @ V; PSUM->DRAM out.
"""

import sys

sys.path.insert(0, "/opt/trn_rl_repo")

import numpy as np
import ml_dtypes

import concourse.mybir as mybir
import concourse.tile as tile
from concourse import bacc
from concourse.bass_utils import run_bass_kernel_spmd

# problem shape (hardcoded per contract)
N_FULL = 16384
D = 2048
H = 8
N_CORES = 8
R = N_FULL // N_CORES  # rows per core = 2048
P = 128
KO = D // P  # 16 k-subtiles along d_in
SL = D // N_CORES  # A rows computed per core = 256
SCALE = 1.0 / np.sqrt(np.float32(D))

BF16 = mybir.dt.bfloat16
F32 = mybir.dt.float32

BLOCKS = [(0, 512), (512, 512), (1024, 512), (1536, 512)]
N_SUB = R // P  # 16 row subtiles
MC = 4  # AllGather column groups of 512
ACH = 8  # a-chunk width 256 -> 8 chunks, 2 m-tiles each
# V chunks after the 257-wide c0 (cols 0:256 + beta): wide chunks keep the
# per-matmul fixed overhead amortized (free=512)
V_CHUNKS = [(256, 512), (768, 512), (1280, 512), (1792, 256)]


def build_program():
    nc = bacc.Bacc("TRN2", target_bir_lowering=False, debug=False, num_devices=N_CORES)

    xs = nc.dram_tensor("xs", [R, D], F32, kind="ExternalInput")
    Wk = nc.dram_tensor("Wk", [D, D], F32, kind="ExternalInput")
    Wv = nc.dram_tensor("Wv", [D, D], F32, kind="ExternalInput")
    wq_sl = nc.dram_tensor("wq_sl", [SL, D], F32, kind="ExternalInput")
    beta_col = nc.dram_tensor("beta_col", [P, KO], F32, kind="ExternalInput")
    bvb = nc.dram_tensor("bvb", [P, D], BF16, kind="ExternalInput")
    maskt = nc.dram_tensor("maskt", [P, P], F32, kind="ExternalInput")
    ident = nc.dram_tensor("ident", [P, P], BF16, kind="ExternalInput")
    ones1 = nc.dram_tensor("ones1", [1, P], BF16, kind="ExternalInput")
    out = nc.dram_tensor("out", [R, D], F32, kind="ExternalOutput")

    # d_in-major view of Wv: w[p, ko, n] = Wv[ko*128+p, n]
    wv_ap = Wv[:].rearrange("(ko p) n -> p ko n", p=P)

    from contextlib import ExitStack

    with tile.TileContext(nc) as tc:
        with ExitStack() as stack:
            pool = lambda *a, **kw: stack.enter_context(tc.tile_pool(*a, **kw))
            const = pool(name="const", bufs=1)
            xT_pool = pool(name="xT", bufs=1)
            big = pool(name="big", bufs=2)  # WkT halves -> uT chunks
            apool = pool(name="achunk", bufs=2)  # wqT_sl -> A chunks
            wvpool = pool(name="wv", bufs=1)  # wv0 only; c1+ use the big pool
            v0pool = pool(name="v0", bufs=1)
            sacc = pool(name="sacc", bufs=1)
            wtmp = pool(name="wtmp", bufs=2)
            wkbp = pool(name="wkb", bufs=2)
            phA = pool(name="phA", bufs=2)
            aoutp = pool(name="aout", bufs=2)
            vpool = pool(name="vpool", bufs=2)
            opool = pool(name="opool", bufs=2)
            soft = pool(name="soft", bufs=2)
            dram = pool(name="dram", bufs=1, space="DRAM")
            ps_big = pool(name="ps_big", bufs=3, space="PSUM")
            ps_s = pool(name="ps_s", bufs=2, space="PSUM")
            ps_t = pool(name="ps_t", bufs=2, space="PSUM")
            ps_warm = pool(name="ps_warm", bufs=1, space="PSUM")
            # --- constants ---
            mask_sb = const.tile([P, P], F32)
            nc.sync.dma_start(mask_sb, maskt[:])
            ident_sb = const.tile([P, P], BF16)
            nc.sync.dma_start(ident_sb, ident[:])
            ones_sb = const.tile([1, P], BF16)
            nc.sync.dma_start(ones_sb, ones1[:])
            bv_sb = const.tile([P, D], BF16)
            nc.sync.dma_start(bv_sb, bvb[:])
            beta_f = const.tile([P, KO], F32)
            nc.sync.dma_start(beta_f, beta_col[:])
            beta_bf = const.tile([P, KO], BF16)
            nc.vector.tensor_copy(beta_bf, beta_f)

            # HAM warm-up: dependency-free matmuls keep the PE clock gate
            # at full rate through the DMA-bound startup window.
            for _ in range(60):
                wps = ps_warm.tile([P, P], F32, tag="warm", name="wps")
                nc.tensor.matmul(wps, lhsT=ident_sb, rhs=ident_sb, start=True, stop=True)

            # --- persistent SBUF intermediates ---
            # xT[bi][p, ko, r] = x[row0 + r, ko*128 + p]  (bf16)
            xT = [
                xT_pool.tile([P, KO, nrows], BF16, name=f"xT{bi}")
                for bi, (_, nrows) in enumerate(BLOCKS)
            ]
            # WkT halves: wkT[h][p, otl, d2] = Wk[d2, (h*8+otl)*128 + p]
            wkT = [big.tile([P, 8, D], BF16, tag="big", name=f"wkT{h}") for h in range(2)]
            # wqT_sl[p, ot, j] = wq_sl[j, ot*128 + p]
            wqT_sl = apool.tile([P, KO, SL], BF16, tag="ach", name="wqT_sl")
            # S accumulator + softmax-weight transpose + b staging
            S_all = sacc.tile([P, N_SUB, P], F32, name="S_all")
            wT_all = sacc.tile([P, N_SUB, P], BF16, name="wT_all")
            b_all = sacc.tile([P, N_SUB], BF16, name="b_all")
            # single-partition row: matmul operands must start at partition 0
            bT_flat = sacc.tile([1, N_SUB * P], BF16, name="bT_flat")
            # V c0 resident tiles (cols 0:256 of V, bias folded in)
            v0_all = v0pool.tile([P, N_SUB, 256], BF16, name="v0_all")

            # DRAM bounce buffers for the A AllGathers
            ag_in = [
                dram.tile([SL, 512], BF16, name=f"ag_in{g}") for g in range(MC)
            ]
            ag_out = [
                dram.tile([D, 512], BF16, addr_space="Shared", name=f"ag_out{g}")
                for g in range(MC)
            ]
            ag_out_ap = [
                ag_out[g][:].rearrange("(kt p) m -> p kt m", p=P) for g in range(MC)
            ]

            # --- Wq slice load + transpose (2MB; DMA'd first) ---
            for j in range(2):
                for s in range(4):
                    xt = phA.tile([P, 512], F32, tag="xt", name="wq_f")
                    nc.sync.dma_start(
                        xt, wq_sl[j * P : (j + 1) * P, s * 512 : (s + 1) * 512]
                    )
                    xb = phA.tile([P, 512], BF16, tag="xb", name="wq_b")
                    nc.vector.tensor_copy(xb, xt)
                    for t in range(4):
                        ot = s * 4 + t
                        pst = ps_t.tile([P, P], BF16, tag="tr", name="pst")
                        nc.tensor.transpose(pst, xb[:, t * P : (t + 1) * P], ident_sb)
                        nc.vector.tensor_copy(
                            wqT_sl[:, ot, j * P : (j + 1) * P], pst
                        )

            # --- Wv c0 chunk (cols 0:256) + beta column -> width 257 ---
            wv0 = wvpool.tile([P, KO, 257], BF16, tag="wv", name="wv0")
            for k0 in range(0, KO, 2):
                tmp = wtmp.tile([P, 2, 256], F32, tag="wtmp", name="wv0_tmp")
                nc.sync.dma_start(tmp, wv_ap[:, k0 : k0 + 2, 0:256])
                nc.vector.tensor_copy(wv0[:, k0 : k0 + 2, 0:256], tmp)
            nc.vector.tensor_copy(wv0[:, :, 256], beta_bf)

            # --- startup quarters: WkT, A-slice chunk, AllGather, xT, V c0 ---
            def wk_quarter(g):
                # 4 slabs of 128 Wk rows -> transposes into wkT
                for sl in range(4):
                    d2_0 = 512 * g + 128 * sl
                    for qq in range(4):
                        wkf = wtmp.tile([P, 2, 256], F32, tag="wtmp", name="wkf")
                        wkf2 = wkf[:].rearrange("p a b -> p (a b)")
                        nc.sync.dma_start(
                            wkf2, Wk[d2_0 : d2_0 + P, qq * 512 : (qq + 1) * 512]
                        )
                        wkb = wkbp.tile([P, 512], BF16, tag="wkb", name="wkb")
                        nc.vector.tensor_copy(wkb, wkf2)
                        for t in range(4):
                            ot = qq * 4 + t
                            pst = ps_t.tile([P, P], BF16, tag="tr", name="pst")
                            nc.tensor.transpose(
                                pst, wkb[:, t * P : (t + 1) * P], ident_sb
                            )
                            nc.vector.tensor_copy(
                                wkT[ot // 8][:, ot % 8, d2_0 : d2_0 + P], pst
                            )

            def a_slice_chunk(g):
                # A[si, 512g:512g+512] = wqT_sl^T @ WkT chunk; bounce + gather
                for db in range(2):
                    psA = ps_big.tile([P, 512], F32, tag="ps_big", name="psA")
                    for ot in range(KO):
                        nc.tensor.matmul(
                            psA,
                            lhsT=wqT_sl[:, ot, db * P : (db + 1) * P],
                            rhs=wkT[ot // 8][:, ot % 8, g * 512 : (g + 1) * 512],
                            start=(ot == 0),
                            stop=(ot == KO - 1),
                        )
                    for hh in range(2):
                        aob = aoutp.tile([P, 256], BF16, tag="aout", name="aob")
                        nc.scalar.activation(
                            aob,
                            psA[:, hh * 256 : (hh + 1) * 256],
                            mybir.ActivationFunctionType.Identity,
                        )
                        nc.sync.dma_start(
                            ag_in[g][db * P : (db + 1) * P, hh * 256 : (hh + 1) * 256],
                            aob,
                        )
                nc.gpsimd.collective_compute(
                    "AllGather",
                    mybir.AluOpType.bypass,
                    replica_groups=[list(range(N_CORES))],
                    ins=[ag_in[g].opt()],
                    outs=[ag_out[g].opt()],
                )

            def phase_a_block(bi):
                row0, nrows = BLOCKS[bi]
                for s in range(4):  # 512-col strips of d_in
                    for rt in range(nrows // P):
                        r0 = row0 + rt * P
                        xt = phA.tile([P, 512], F32, tag="xt", name="xt")
                        nc.sync.dma_start(
                            xt, xs[r0 : r0 + P, s * 512 : (s + 1) * 512]
                        )
                        xb = phA.tile([P, 512], BF16, tag="xb", name="xb")
                        nc.vector.tensor_copy(xb, xt)
                        for t in range(4):
                            kt = s * 4 + t
                            pst = ps_t.tile([P, P], BF16, tag="tr", name="pst")
                            nc.tensor.transpose(
                                pst, xb[:, t * P : (t + 1) * P], ident_sb
                            )
                            nc.vector.tensor_copy(
                                xT[bi][:, kt, rt * P : (rt + 1) * P], pst
                            )

            def v0_block(bi):
                # V c0 (+bias) and the per-row b = x.beta (psum col 256)
                for rs in range(4):
                    i = bi * 4 + rs
                    psv = ps_big.tile([P, 512], F32, tag="ps_big", name="psv")[:, :257]
                    for kt in range(KO):
                        nc.tensor.matmul(
                            psv,
                            lhsT=xT[bi][:, kt, rs * P : (rs + 1) * P],
                            rhs=wv0[:, kt, :],
                            start=(kt == 0),
                            stop=(kt == KO - 1),
                        )
                    nc.vector.tensor_copy(b_all[:, i : i + 1], psv[:, 256:257])
                    nc.vector.tensor_add(
                        v0_all[:, i, :], psv[:, 0:256], bv_sb[:, 0:256]
                    )

            # ring-order note: phase_a_block(g) is emitted before
            # a_slice_chunk(g) so the A-bounce DMA (gated on the A-chunk
            # matmuls) sits behind the x loads in the sync DMA ring.
            for g in range(MC):
                wk_quarter(g)
                phase_a_block(g)
                a_slice_chunk(g)
                v0_block(g)

            # b columns -> partition-0 rows (rhs for the rank-1 bias matmul)
            for i in range(N_SUB):
                pst = ps_t.tile([P, P], BF16, tag="tr", name="pst_b")
                nc.tensor.transpose(
                    pst[:1, :], b_all[:, i : i + 1], ident_sb
                )
                nc.vector.tensor_copy(
                    bT_flat[:, i * P : (i + 1) * P], pst[:1, :]
                )

            # --- pass u: uT GEMM + S partial accumulation ---
            def load_a_chunk(ach):
                g, half = ach // 2, ach % 2
                dst = apool.tile([P, KO, 256], BF16, tag="ach", name="a_sb")
                nc.sync.dma_start(
                    dst, ag_out_ap[g][:, :, half * 256 : (half + 1) * 256]
                )
                return dst

            a_tiles = {0: load_a_chunk(0), 1: load_a_chunk(1)}
            uT = [None, None]  # big-pool slots, 8 m-tiles each

            def emit_s_pair(ach, i0):
                # S partials for subtiles i0, i0+1 of chunk `ach` (2 m-tiles);
                # the last chunk's chain appends the rank-1 ones (x) b^T term.
                uh = ach // 4
                last = ach == ACH - 1
                for i in (i0, i0 + 1):
                    bi, rs = i // 4, i % 4
                    pss = ps_s.tile([P, P], F32, tag="pss", name="pss")
                    for ml in range(2):
                        m_local = (ach % 4) * 2 + ml
                        mt = ach * 2 + ml
                        nc.tensor.matmul(
                            pss,
                            lhsT=uT[uh][:, m_local, i * P : (i + 1) * P],
                            rhs=xT[bi][:, mt, rs * P : (rs + 1) * P],
                            start=(ml == 0),
                            stop=(ml == 1 and not last),
                        )
                    if last:
                        nc.tensor.matmul(
                            pss,
                            lhsT=ones_sb,
                            rhs=bT_flat[:, i * P : (i + 1) * P],
                            start=False,
                            stop=True,
                        )
                    if ach == 0:
                        nc.vector.tensor_copy(S_all[:, i, :], pss)
                    else:
                        nc.vector.tensor_add(S_all[:, i, :], S_all[:, i, :], pss)

            # u-chains for chunk `ach` interleave with the S partials of
            # chunk `ach-1`, so the short S chains and their PSUM->DVE
            # drains hide behind the long (free=512) u matmuls.
            pending_s = None
            for ach in range(ACH):
                if ach + 2 < ACH:
                    a_tiles[ach + 2] = load_a_chunk(ach + 2)
                a_sb = a_tiles.pop(ach)
                uh = ach // 4  # which uT half
                if ach % 4 == 0:
                    uT[uh] = big.tile([P, 8, R], BF16, tag="big", name=f"uT{uh}")
                for j in range(8):  # 2 m-tiles x 4 row blocks
                    ml, bi = j // 4, j % 4
                    row0, nrows = BLOCKS[bi]
                    m_local = (ach % 4) * 2 + ml
                    psu = ps_big.tile([P, 512], F32, tag="ps_big", name="psu")
                    for kt in range(KO):
                        nc.tensor.matmul(
                            psu,
                            lhsT=a_sb[:, kt, ml * P : (ml + 1) * P],
                            rhs=xT[bi][:, kt, :],
                            start=(kt == 0),
                            stop=(kt == KO - 1),
                        )
                    nc.scalar.activation(
                        uT[uh][:, m_local, row0 : row0 + nrows],
                        psu,
                        mybir.ActivationFunctionType.Identity,
                    )
                    if pending_s is not None:
                        emit_s_pair(pending_s, 2 * j)
                pending_s = ach

            # --- softmax + O c0 (tail S chains of the last chunk interleave) ---
            def emit_softmax(i):
                tmask = soft.tile([P, P], F32, tag="tmask")
                nc.vector.tensor_add(tmask, S_all[:, i, :], mask_sb)
                e = soft.tile([P, P], F32, tag="e")
                ssum = soft.tile([P, 1], F32, tag="ssum")
                nc.scalar.activation(
                    e, tmask, mybir.ActivationFunctionType.Exp,
                    scale=float(SCALE), accum_out=ssum,
                )
                rcp = soft.tile([P, 1], F32, tag="rcp")
                nc.vector.reciprocal(rcp, ssum)
                wsb = soft.tile([P, P], BF16, tag="wsb")
                nc.vector.tensor_scalar_mul(wsb, e, rcp)
                pstw = ps_t.tile([P, P], BF16, tag="tr", name="pstw")
                nc.tensor.transpose(pstw, wsb, ident_sb)
                nc.vector.tensor_copy(wT_all[:, i, :], pstw)

            # last chunk's S chains pipelined against the softmaxes
            for p in range(8):
                emit_s_pair(ACH - 1, 2 * p)
                if p >= 1:
                    emit_softmax(2 * (p - 1))
                    emit_softmax(2 * (p - 1) + 1)
            pending_s = None
            emit_softmax(14)
            emit_softmax(15)
            for i in range(N_SUB):
                pso = ps_big.tile([P, 512], F32, tag="ps_big", name="pso0")[:, :256]
                nc.tensor.matmul(
                    pso, lhsT=wT_all[:, i, :], rhs=v0_all[:, i, :],
                    start=True, stop=True,
                )
                o_sb = opool.tile([P, 512], F32, tag="o", name="o_sb0")[:, :256]
                nc.vector.tensor_copy(o_sb, pso)
                nc.sync.dma_start(out[i * P : (i + 1) * P, 0:256], o_sb)

            # --- pass 2: V chunks c1..c4 + O ---
            # chunk tiles live in the big pool: slots rotate out of the dead
            # uT halves (their last readers are the S partial chains)
            def load_wv_chunk(c):
                col0, width = V_CHUNKS[c]
                dst = big.tile([P, KO, 512], BF16, tag="big", name="wv_sb")
                for k0 in range(KO):
                    tmp = wtmp.tile([P, 1, 512], F32, tag="wtmp", name="wv_tmp")[
                        :, :, :width
                    ]
                    nc.sync.dma_start(
                        tmp, wv_ap[:, k0 : k0 + 1, col0 : col0 + width]
                    )
                    nc.vector.tensor_copy(dst[:, k0 : k0 + 1, 0:width], tmp)
                return dst

            wv_tiles = {0: load_wv_chunk(0)}
            pending_o = None  # (v_sb, i, col0, width)

            def emit_o(v_sb, i, col0, width):
                pso = ps_big.tile([P, 512], F32, tag="ps_big", name="pso")[:, :width]
                nc.tensor.matmul(
                    pso, lhsT=wT_all[:, i, :], rhs=v_sb, start=True, stop=True
                )
                o_sb = opool.tile([P, 512], F32, tag="o", name="o_sb")[:, :width]
                nc.vector.tensor_copy(o_sb, pso)
                nc.sync.dma_start(out[i * P : (i + 1) * P, col0 : col0 + width], o_sb)

            for c in range(len(V_CHUNKS)):
                col0, width = V_CHUNKS[c]
                if c + 1 < len(V_CHUNKS) and (c + 1) not in wv_tiles:
                    wv_tiles[c + 1] = load_wv_chunk(c + 1)
                wv_sb = wv_tiles.pop(c)
                for bi, (row0, nrows) in enumerate(BLOCKS):
                    for rs in range(4):
                        i = bi * 4 + rs
                        psv = ps_big.tile([P, 512], F32, tag="ps_big", name="psv2")[
                            :, :width
                        ]
                        for kt in range(KO):
                            nc.tensor.matmul(
                                psv,
                                lhsT=xT[bi][:, kt, rs * P : (rs + 1) * P],
                                rhs=wv_sb[:, kt, 0:width],
                                start=(kt == 0),
                                stop=(kt == KO - 1),
                            )
                        v_sb = vpool.tile([P, 512], BF16, tag="v", name="v_sb")[
                            :, :width
                        ]
                        nc.vector.tensor_add(
                            v_sb, psv, bv_sb[:, col0 : col0 + width]
                        )
                        if pending_o is not None:
                            emit_o(*pending_o)
                        pending_o = (v_sb, i, col0, width)
            if pending_o is not None:
                emit_o(*pending_o)
                pending_o = None

    nc.compile()
    return nc


_CACHED = {}


def host_constants():
    mask = np.full((P, P), -1e9, dtype=np.float32)
    for g in range(P // H):
        mask[g * H : (g + 1) * H, g * H : (g + 1) * H] = 0.0
    identity = np.eye(P, dtype=ml_dtypes.bfloat16)
    ones_row = np.ones((1, P), dtype=ml_dtypes.bfloat16)
    return mask, identity, ones_row


def make_in_maps(x, Wq, bq, Wk, bk, Wv, bv):
    x = np.ascontiguousarray(np.asarray(x, dtype=np.float32))
    Wq = np.ascontiguousarray(np.asarray(Wq, dtype=np.float32))
    Wk = np.ascontiguousarray(np.asarray(Wk, dtype=np.float32))
    Wv = np.ascontiguousarray(np.asarray(Wv, dtype=np.float32))
    bq = np.asarray(bq, dtype=np.float32)
    bv = np.asarray(bv, dtype=np.float32)

    mask, identity, ones_row = host_constants()
    beta = Wk @ bq  # [D]; the surviving score-bias term is b = x @ beta
    beta_col = np.ascontiguousarray(beta.reshape(KO, P).T)
    bvb = np.ascontiguousarray(
        np.broadcast_to(bv.astype(ml_dtypes.bfloat16), (P, D))
    )

    in_maps = []
    for i in range(N_CORES):
        in_maps.append(
            {
                "xs": x[i * R : (i + 1) * R],
                "Wk": Wk,
                "Wv": Wv,
                "wq_sl": np.ascontiguousarray(Wq[i * SL : (i + 1) * SL]),
                "beta_col": beta_col,
                "bvb": bvb,
                "maskt": mask,
                "ident": identity,
                "ones1": ones_row,
            }
        )
    return in_maps


def kernel(x, Wq, bq, Wk, bk, Wv, bv):
    if "nc" not in _CACHED:
        _CACHED["nc"] = build_program()
    nc = _CACHED["nc"]
    in_maps = make_in_maps(x, Wq, bq, Wk, bk, Wv, bv)
    res = run_bass_kernel_spmd(nc, in_maps, list(range(N_CORES)))
    return np.concatenate([res.results[i]["out"] for i in range(N_CORES)], axis=0)
